# revision 1
# baseline (speedup 1.0000x reference)
"""Conv5d (nn_Conv5d_36206574306083) Bass kernel for 8 trn2 NeuronCores.

Math: out[b,o,c,t] = (1/9) * sum_{i,j in 0..2} Conv3d_{ij}(x[b,:,c+i,t+j]) + mean_bias
with x [2,4,8,8,8,96,96], W [9,4,4,3,3,3], b [9,4].

Mapping: data-parallel over (b, c-group, t-group) -> 8 cores. Per core the
inner 5D conv is computed as PSUM-accumulated banded matmuls:
  stationary S[(hslot4, ci4, d8) = 128, (o4, d8, hh2) = 64]  (27 of them,
  one per (i, j, kw) round; kd and kh taps live inside the band)
  moving rhs = pre-tiled x window [128, (c3, t3, w48)]
Two h-pair chains interleave into PSUM partition halves 0:64 / 64:128 so the
PE weight loads of one chain hide under the other chain's streaming (bf16
mode). float32r mode keeps separate [64, N] chains (slower, higher precision).
"""
import os
import sys

sys.path.insert(0, '/opt/trn_rl_repo')

import numpy as np

# ---------------------------------------------------------------- constants
B, C, CD, T, D, H, WD = 2, 4, 8, 8, 8, 96, 96
O = 4
CC, TT = CD - 2, T - 2          # 6, 6 output c/t positions
CPC, TPC = CC // 2, TT // 2     # 3, 3 outputs per core in c and t
NCORES = 8
HB = 24                         # h blocks of 4 output rows
WH = 2                          # w halves of 48
NACC = 27                       # rounds per chain: (i, j, kw)
KP = 128                        # contraction partitions (slot4, ci4, d8)
MP = 64                         # stationary cols (o4, d8, hh2)
FREE = 3 * 3 * 48               # 432 = (c3, t3, w48)
XF = 5 * 5 * 98                 # 2450 free elems per x tile (cdim5, td5, w98)

MODE = os.environ.get("CONV_MODE", "bf16")  # "bf16" | "f32r"

_CACHE = {}


def _install_ntff_hook():
    """Optional: lets run_bass_kernel_spmd(trace=True) profile under axon."""
    import types
    name = 'antenv.axon_hooks'
    if name in sys.modules:
        return
    try:
        import antenv
        mod = types.ModuleType(name)
        mod._hook = None
        mod.set_axon_ntff_profile_hook = lambda h: setattr(mod, '_hook', h)
        mod.get_axon_ntff_profile_hook = lambda: mod._hook
        sys.modules[name] = mod
        antenv.axon_hooks = mod
        from trn_agent_boot.trn_boot import _ntff_profile_via_ctypes
        hook = _ntff_profile_via_ctypes('/opt/axon/libaxon_pjrt.so')
        if hook is not None:
            mod._hook = hook
    except Exception:
        pass


def _build(mode):
    import concourse.bacc as bacc
    import concourse.mybir as mybir
    from concourse.tile import TileContext

    mdt = mybir.dt.bfloat16 if mode == "bf16" else mybir.dt.float32r
    f32 = mybir.dt.float32

    nc = bacc.Bacc("TRN2", target_bir_lowering=False, debug=False,
                   num_devices=NCORES)
    xs = nc.dram_tensor("xs", [HB, 2, KP, XF], mdt, kind="ExternalInput").ap()
    stat = nc.dram_tensor("stat", [NACC, KP, MP], mdt,
                          kind="ExternalInput").ap()
    bias = nc.dram_tensor("bias", [2 * MP, FREE], f32,
                          kind="ExternalInput").ap()
    out = nc.dram_tensor("out", [HB, WH, 2 * MP, FREE], f32,
                         kind="ExternalOutput").ap()

    # round order: a = (i*3 + j)*3 + kw
    ijkw = [(i, j, kw)
            for i in range(3) for j in range(3) for kw in range(3)]

    with TileContext(nc) as tc:
        with (tc.tile_pool(name="const", bufs=1) as cp,
              tc.tile_pool(name="xt", bufs=4) as xp,
              tc.tile_pool(name="ps", bufs=(6 if mode == "bf16" else 3), space="PSUM") as pp,
              tc.tile_pool(name="ot", bufs=4) as op):
            st = cp.tile([KP, NACC * MP], mdt)
            for a in range(NACC):
                nc.sync.dma_start(out=st[:, a * MP:(a + 1) * MP],
                                  in_=stat[a])
            bt = cp.tile([2 * MP, FREE], f32)
            nc.sync.dma_start(out=bt[:], in_=bias[:])

            for hb in range(HB):
                ta = xp.tile([KP, XF], mdt, tag="xa")
                tb = xp.tile([KP, XF], mdt, tag="xb")
                nc.sync.dma_start(out=ta[:], in_=xs[hb, 0])
                nc.sync.dma_start(out=tb[:], in_=xs[hb, 1])
                for wh in range(WH):
                    base = wh * 48
                    if mode == "bf16":
                        ps = pp.tile([2 * MP, FREE], f32)
                        for a in range(NACC):
                            i, j, kw = ijkw[a]
                            rhs_a = ta[:].rearrange(
                                "k (c t w) -> k c t w", c=5, t=5)[
                                :, i:i + 3, j:j + 3,
                                base + kw:base + kw + 48]
                            rhs_b = tb[:].rearrange(
                                "k (c t w) -> k c t w", c=5, t=5)[
                                :, i:i + 3, j:j + 3,
                                base + kw:base + kw + 48]
                            nc.tensor.matmul(
                                ps[0:MP, :], st[:, a * MP:(a + 1) * MP], rhs_a,
                                start=(a == 0), stop=(a == NACC - 1))
                            nc.tensor.matmul(
                                ps[MP:2 * MP, :], st[:, a * MP:(a + 1) * MP],
                                rhs_b,
                                start=(a == 0), stop=(a == NACC - 1))
                        ot = op.tile([2 * MP, FREE], f32)
                        nc.vector.tensor_add(ot[:], ps[:], bt[:])
                        nc.sync.dma_start(out=out[hb, wh], in_=ot[:])
                    else:
                        psa = pp.tile([MP, FREE], f32, tag="psa")
                        psb = pp.tile([MP, FREE], f32, tag="psb")
                        for a in range(NACC):
                            i, j, kw = ijkw[a]
                            rhs_a = ta[:].rearrange(
                                "k (c t w) -> k c t w", c=5, t=5)[
                                :, i:i + 3, j:j + 3,
                                base + kw:base + kw + 48]
                            rhs_b = tb[:].rearrange(
                                "k (c t w) -> k c t w", c=5, t=5)[
                                :, i:i + 3, j:j + 3,
                                base + kw:base + kw + 48]
                            nc.tensor.matmul(
                                psa[:], st[:, a * MP:(a + 1) * MP], rhs_a,
                                start=(a == 0), stop=(a == NACC - 1))
                            nc.tensor.matmul(
                                psb[:], st[:, a * MP:(a + 1) * MP], rhs_b,
                                start=(a == 0), stop=(a == NACC - 1))
                        ot = op.tile([2 * MP, FREE], f32)
                        nc.vector.tensor_add(ot[0:MP, :], psa[:], bt[0:MP, :])
                        nc.vector.tensor_add(ot[MP:2 * MP, :], psb[:],
                                             bt[MP:2 * MP, :])
                        nc.sync.dma_start(out=out[hb, wh], in_=ot[:])

    nc.compile()
    return nc


def _host_prep(x, Wk, b, mode):
    """Build per-core input maps (pre-tiled x windows, stationaries, bias)."""
    if mode == "bf16":
        import ml_dtypes
        npdt = ml_dtypes.bfloat16
    else:
        npdt = np.float32

    # stationaries: S[a, (slot,ci,d'), (o,d,hh)]
    S = np.zeros((NACC, 4, C, D, O, D, 2), np.float32)
    for i in range(3):
        for j in range(3):
            for kw in range(3):
                a = (i * 3 + j) * 3 + kw
                for slot in range(4):
                    for hh in range(2):
                        kh = slot - hh
                        if not 0 <= kh <= 2:
                            continue
                        for d in range(D):
                            for kd in range(3):
                                dp = d + kd - 1
                                if not 0 <= dp < D:
                                    continue
                                S[a, slot, :, dp, :, d, hh] = \
                                    Wk[i * 3 + j, :, :, kd, kh, kw].T / 9.0
    S = S.reshape(NACC, KP, MP).astype(npdt)

    mean_b = (b.sum(0) / 9.0).astype(np.float32)
    bias_full = np.empty((2 * MP, FREE), np.float32)
    for p in range(2 * MP):
        bias_full[p, :] = mean_b[(p % MP) // 16]

    in_maps = []
    for core in range(NCORES):
        bb, cg, tg = core // 4, (core // 2) % 2, core % 2
        xsh = x[bb, :, cg * 3:cg * 3 + 5, tg * 3:tg * 3 + 5]  # [4,5,5,8,96,96]
        xpad = np.zeros((C, 5, 5, D, H + 2, WD + 2), np.float32)
        xpad[:, :, :, :, 1:H + 1, 1:WD + 1] = xsh
        # xs[hb, ab, (slot,ci,d), (cdim,td,w)]; window rows h'=4hb-1+2ab+slot
        xt = np.empty((HB, 2, 4, C, D, 5, 5, WD + 2), np.float32)
        for hb in range(HB):
            for ab in range(2):
                h0 = 4 * hb + 2 * ab  # xpad h index of slot 0 (= x h' - 1 + 1)
                blk = xpad[:, :, :, :, h0:h0 + 4, :]     # ci,cd,td,d,slot,w
                xt[hb, ab] = blk.transpose(4, 0, 3, 1, 2, 5)
        in_maps.append({
            "xs": xt.reshape(HB, 2, KP, XF).astype(npdt),
            "stat": S,
            "bias": bias_full,
        })
    return in_maps


def kernel(x, W, b, trace=False):
    x = np.asarray(x, np.float32)
    W = np.asarray(W, np.float32)
    b = np.asarray(b, np.float32)

    mode = MODE
    if mode not in _CACHE:
        _install_ntff_hook()
        _CACHE[mode] = _build(mode)
    nc = _CACHE[mode]

    from concourse.bass_utils import run_bass_kernel_spmd
    in_maps = _host_prep(x, W, b, mode)
    res = run_bass_kernel_spmd(nc, in_maps, core_ids=list(range(NCORES)),
                               trace=trace)
    kernel.last_exec_ns = res.exec_time_ns

    outf = np.empty((B, O, CC, TT, D, H, WD), np.float32)
    for core in range(NCORES):
        bb, cg, tg = core // 4, (core // 2) % 2, core % 2
        r = res.results[core]["out"]  # [HB, WH, 128, FREE]
        r = r.reshape(HB, WH, 2, O, D, 2, 3, 3, 48)
        # dims: hb, wh, hp, o, d, hh, c, t, wc -> o c t d (hb hp hh) (wh wc)
        r = r.transpose(3, 6, 7, 4, 0, 2, 5, 1, 8)
        r = r.reshape(O, 3, 3, D, H, WD)
        outf[bb, :, cg * 3:cg * 3 + 3, tg * 3:tg * 3 + 3] = r
    return outf


kernel.last_exec_ns = None



# revision 7
# speedup vs baseline: 1.1556x; 1.1556x over previous
"""Conv5d (nn_Conv5d_36206574306083) Bass kernel for 8 trn2 NeuronCores.

Math: out[b,o,c,t] = (1/9) * sum_{i,j in 0..2} Conv3d_{ij}(x[b,:,c+i,t+j]) + mean_bias
with x [2,4,8,8,8,96,96], W [9,4,4,3,3,3], b [9,4].

Mapping: data-parallel over (b, c-group, t-group) -> 8 cores. Per core the
inner 5D conv is computed as PSUM-accumulated banded matmuls:
  stationary S[(hslot4, ci4, d8) = 128, (o4, d8, hh2) = 64]  (27 of them,
  one per (i, j, kw) round; kd and kh taps live inside the band)
  moving rhs = pre-tiled x window [128, (c3, t3, w48)]
Two h-pair chains interleave into PSUM partition halves 0:64 / 64:128 so the
PE weight loads of one chain hide under the other chain's streaming (bf16
mode). float32r mode keeps separate [64, N] chains (slower, higher precision).
"""
import os
import sys

sys.path.insert(0, '/opt/trn_rl_repo')

import numpy as np

# ---------------------------------------------------------------- constants
B, C, CD, T, D, H, WD = 2, 4, 8, 8, 8, 96, 96
O = 4
CC, TT = CD - 2, T - 2          # 6, 6 output c/t positions
CPC, TPC = CC // 2, TT // 2     # 3, 3 outputs per core in c and t
NCORES = 8
HB = 24                         # h blocks of 4 output rows
WH = 2                          # w halves of 48
NACC = 27                       # rounds per chain: (i, j, kw)
KP = 128                        # contraction partitions (slot4, ci4, d8)
MP = 64                         # stationary cols (o4, d8, hh2)
FREE = 3 * 3 * 48               # 432 = (c3, t3, w48)
XF = 5 * 5 * 98                 # 2450 free elems per x tile (cdim5, td5, w98)

MODE = os.environ.get("CONV_MODE", "bf16")  # "bf16" | "f32r"

_CACHE = {}


def _install_ntff_hook():
    """Optional: lets run_bass_kernel_spmd(trace=True) profile under axon."""
    import types
    name = 'antenv.axon_hooks'
    if name in sys.modules:
        return
    try:
        import antenv
        mod = types.ModuleType(name)
        mod._hook = None
        mod.set_axon_ntff_profile_hook = lambda h: setattr(mod, '_hook', h)
        mod.get_axon_ntff_profile_hook = lambda: mod._hook
        sys.modules[name] = mod
        antenv.axon_hooks = mod
        from trn_agent_boot.trn_boot import _ntff_profile_via_ctypes
        hook = _ntff_profile_via_ctypes('/opt/axon/libaxon_pjrt.so')
        if hook is not None:
            mod._hook = hook
    except Exception:
        pass


def _build(mode):
    import concourse.bacc as bacc
    import concourse.mybir as mybir
    from concourse.tile import TileContext

    mdt = mybir.dt.bfloat16 if mode == "bf16" else mybir.dt.float32r
    f32 = mybir.dt.float32

    nc = bacc.Bacc("TRN2", target_bir_lowering=False, debug=False,
                   num_devices=NCORES)
    xs = nc.dram_tensor("xs", [HB, 2, KP, XF], mdt, kind="ExternalInput").ap()
    stat = nc.dram_tensor("stat", [NACC, KP, MP], mdt,
                          kind="ExternalInput").ap()
    bias = nc.dram_tensor("bias", [2 * MP, FREE], f32,
                          kind="ExternalInput").ap()
    out = nc.dram_tensor("out", [HB, WH, 2 * MP, FREE], f32,
                         kind="ExternalOutput").ap()

    # round order: a = (i*3 + j)*3 + kw
    ijkw = [(i, j, kw)
            for i in range(3) for j in range(3) for kw in range(3)]

    G = 3                      # hb per group; 2*G psum tiles live per group
    with TileContext(nc) as tc:
        with (tc.tile_pool(name="const", bufs=1) as cp,
              tc.tile_pool(name="xt", bufs=12) as xp,
              tc.tile_pool(name="ps", bufs=8, space="PSUM") as pp,
              tc.tile_pool(name="ot", bufs=4) as op):
            st = cp.tile([KP, NACC * MP], mdt)
            for a in range(NACC):
                nc.sync.dma_start(out=st[:, a * MP:(a + 1) * MP],
                                  in_=stat[a])
            bt = cp.tile([2 * MP, FREE], f32)
            nc.sync.dma_start(out=bt[:], in_=bias[:])

            for g in range(HB // G):
                tiles = []
                for hg in range(G):
                    hb = g * G + hg
                    ta = xp.tile([KP, XF], mdt, tag="x", name=f"xa{hg}")
                    tb = xp.tile([KP, XF], mdt, tag="x", name=f"xb{hg}")
                    nc.sync.dma_start(out=ta[:], in_=xs[hb, 0])
                    nc.sync.dma_start(out=tb[:], in_=xs[hb, 1])
                    tiles.append((hb, ta, tb))
                pst = [[pp.tile([2 * MP, FREE], f32, tag="ps",
                                name=f"ps{hg}{wh}")
                        for wh in range(WH)] for hg in range(G)]
                # round-major: one stationary feeds all 2*G psum tiles
                for a in range(NACC):
                    i, j, kw = ijkw[a]
                    sta = st[:, a * MP:(a + 1) * MP]
                    for hg, (hb, ta, tb) in enumerate(tiles):
                        for wh in range(WH):
                            base = wh * 48
                            rhs_a = ta[:].rearrange(
                                "k (c t w) -> k c t w", c=5, t=5)[
                                :, i:i + 3, j:j + 3,
                                base + kw:base + kw + 48]
                            rhs_b = tb[:].rearrange(
                                "k (c t w) -> k c t w", c=5, t=5)[
                                :, i:i + 3, j:j + 3,
                                base + kw:base + kw + 48]
                            ps = pst[hg][wh]
                            nc.tensor.matmul(
                                ps[0:MP, :], sta, rhs_a,
                                start=(a == 0), stop=(a == NACC - 1))
                            nc.tensor.matmul(
                                ps[MP:2 * MP, :], sta, rhs_b,
                                start=(a == 0), stop=(a == NACC - 1))
                for hg, (hb, ta, tb) in enumerate(tiles):
                    for wh in range(WH):
                        ot = op.tile([2 * MP, FREE], f32)
                        nc.vector.tensor_add(ot[:], pst[hg][wh][:], bt[:])
                        nc.sync.dma_start(out=out[hb, wh], in_=ot[:])

    nc.compile()
    return nc


def _host_prep(x, Wk, b, mode):
    """Build per-core input maps (pre-tiled x windows, stationaries, bias)."""
    if mode == "bf16":
        import ml_dtypes
        npdt = ml_dtypes.bfloat16
    else:
        npdt = np.float32

    # stationaries: S[a, (slot,ci,d'), (o,d,hh)]
    S = np.zeros((NACC, 4, C, D, O, D, 2), np.float32)
    for i in range(3):
        for j in range(3):
            for kw in range(3):
                a = (i * 3 + j) * 3 + kw
                for slot in range(4):
                    for hh in range(2):
                        kh = slot - hh
                        if not 0 <= kh <= 2:
                            continue
                        for d in range(D):
                            for kd in range(3):
                                dp = d + kd - 1
                                if not 0 <= dp < D:
                                    continue
                                S[a, slot, :, dp, :, d, hh] = \
                                    Wk[i * 3 + j, :, :, kd, kh, kw].T / 9.0
    S = S.reshape(NACC, KP, MP).astype(npdt)

    mean_b = (b.sum(0) / 9.0).astype(np.float32)
    bias_full = np.empty((2 * MP, FREE), np.float32)
    for p in range(2 * MP):
        bias_full[p, :] = mean_b[(p % MP) // 16]

    in_maps = []
    for core in range(NCORES):
        bb, cg, tg = core // 4, (core // 2) % 2, core % 2
        xsh = x[bb, :, cg * 3:cg * 3 + 5, tg * 3:tg * 3 + 5]  # [4,5,5,8,96,96]
        xpad = np.zeros((C, 5, 5, D, H + 2, WD + 2), np.float32)
        xpad[:, :, :, :, 1:H + 1, 1:WD + 1] = xsh
        # xs[hb, ab, (slot,ci,d), (cdim,td,w)]; window rows h'=4hb-1+2ab+slot
        xt = np.empty((HB, 2, 4, C, D, 5, 5, WD + 2), np.float32)
        for hb in range(HB):
            for ab in range(2):
                h0 = 4 * hb + 2 * ab  # xpad h index of slot 0 (= x h' - 1 + 1)
                blk = xpad[:, :, :, :, h0:h0 + 4, :]     # ci,cd,td,d,slot,w
                xt[hb, ab] = blk.transpose(4, 0, 3, 1, 2, 5)
        in_maps.append({
            "xs": xt.reshape(HB, 2, KP, XF).astype(npdt),
            "stat": S,
            "bias": bias_full,
        })
    return in_maps


def kernel(x, W, b, trace=False):
    x = np.asarray(x, np.float32)
    W = np.asarray(W, np.float32)
    b = np.asarray(b, np.float32)

    mode = MODE
    if mode not in _CACHE:
        _install_ntff_hook()
        _CACHE[mode] = _build(mode)
    nc = _CACHE[mode]

    from concourse.bass_utils import run_bass_kernel_spmd
    in_maps = _host_prep(x, W, b, mode)
    res = run_bass_kernel_spmd(nc, in_maps, core_ids=list(range(NCORES)),
                               trace=trace)
    kernel.last_exec_ns = res.exec_time_ns

    outf = np.empty((B, O, CC, TT, D, H, WD), np.float32)
    for core in range(NCORES):
        bb, cg, tg = core // 4, (core // 2) % 2, core % 2
        r = res.results[core]["out"]  # [HB, WH, 128, FREE]
        r = r.reshape(HB, WH, 2, O, D, 2, 3, 3, 48)
        # dims: hb, wh, hp, o, d, hh, c, t, wc -> o c t d (hb hp hh) (wh wc)
        r = r.transpose(3, 6, 7, 4, 0, 2, 5, 1, 8)
        r = r.reshape(O, 3, 3, D, H, WD)
        outf[bb, :, cg * 3:cg * 3 + 3, tg * 3:tg * 3 + 3] = r
    return outf


kernel.last_exec_ns = None

